# revision 16
# baseline (speedup 1.0000x reference)
"""Trainium2 Bass kernel for a cross-attention block with position-routed MoE.

Contract: kernel(**inputs) takes the FULL fp32 inputs (as produced by
setup_inputs) and returns the FULL [4, 1024, 1024] fp32 output.

Sharding (8 cores): core c handles batch b = c//2 and query-token half
h = c%2 (512 tokens). Tokens are permuted on the host so they are grouped
by position%4; MoE expert e then corresponds to contiguous token tile e.
KV projections are recomputed per half (duplicated across the 2 cores of a
batch) to avoid any cross-core communication.

Schedule: attention head-pairs are interleaved with the k-projection so the
ScalarE exp work overlaps the PE-dense projection stream; MoE weights are
prefetched at the attention/MoE boundary.
"""

import sys

if "/opt/trn_rl_repo" not in sys.path:
    sys.path.insert(0, "/opt/trn_rl_repo")

import numpy as np
import ml_dtypes

B = 4
NQ = 1024
NKV = 2048
H = 1024
NH = 16
D = 64  # head dim
E = 4
I = 1024  # expert intermediate
T = 512  # q tokens per core
P = 128
EPS = 1e-6
KK = H // P  # 8 contraction tiles
NST = NKV // P  # 16 kv-token tiles
NQT = T // P  # 4 q-token tiles

_BUILT = {}


def _build_program():
    from contextlib import ExitStack

    from concourse import bacc
    import concourse.mybir as mybir
    import concourse.tile as tile
    from concourse.masks import make_identity

    bf16 = mybir.dt.bfloat16
    f32 = mybir.dt.float32
    Alu = mybir.AluOpType
    Act = mybir.ActivationFunctionType

    nc = bacc.Bacc("TRN2", target_bir_lowering=False, debug=False, num_devices=8)

    # ---- DRAM I/O ----
    q_d = nc.dram_tensor("q", [T, H], f32, kind="ExternalInput")
    kvT_d = nc.dram_tensor("kvT", [H, NKV], bf16, kind="ExternalInput")
    wq_d = nc.dram_tensor("wq", [H, H], bf16, kind="ExternalInput")
    wk_d = nc.dram_tensor("wk", [H, H], bf16, kind="ExternalInput")
    wv_d = nc.dram_tensor("wv", [H, H], bf16, kind="ExternalInput")
    wo_d = nc.dram_tensor("wo", [H, H], bf16, kind="ExternalInput")
    bq_d = nc.dram_tensor("bq", [H], f32, kind="ExternalInput")
    bk_d = nc.dram_tensor("bk", [H], f32, kind="ExternalInput")
    bv_d = nc.dram_tensor("bv", [H], f32, kind="ExternalInput")
    bo_d = nc.dram_tensor("bo", [H], f32, kind="ExternalInput")
    g1_d = nc.dram_tensor("g1", [H], f32, kind="ExternalInput")
    b1_d = nc.dram_tensor("b1", [H], f32, kind="ExternalInput")
    g2_d = nc.dram_tensor("g2", [H], f32, kind="ExternalInput")
    b2_d = nc.dram_tensor("b2", [H], f32, kind="ExternalInput")
    gup_d = nc.dram_tensor("gup", [E, H, 2 * I], bf16, kind="ExternalInput")
    dwn_d = nc.dram_tensor("dwn", [E, I, H], bf16, kind="ExternalInput")
    out_d = nc.dram_tensor("out", [T, H], f32, kind="ExternalOutput")

    with tile.TileContext(nc) as tc, ExitStack() as stk:
        # ---- persistent pools (~22 KB/partition) ----
        consts = stk.enter_context(tc.tile_pool(name="consts", bufs=1))
        bcast = stk.enter_context(tc.tile_pool(name="bcast", bufs=3))
        lnp = stk.enter_context(tc.tile_pool(name="lnp", bufs=2))
        xnp = stk.enter_context(tc.tile_pool(name="xnp", bufs=2))
        xnTp = stk.enter_context(tc.tile_pool(name="xnT", bufs=8))
        psA = stk.enter_context(tc.tile_pool(name="psA", bufs=2, space="PSUM"))
        psB = stk.enter_context(tc.tile_pool(name="psB", bufs=2, space="PSUM"))

        ident = consts.tile([P, P], bf16, tag="ident")
        make_identity(nc, ident)
        eps_t = consts.tile([P, 1], f32, tag="eps")
        nc.vector.memset(eps_t, EPS)
        bq_t = consts.tile([P, KK], f32, tag="bq")
        bk_t = consts.tile([P, KK], f32, tag="bk")

        def bcast_tile(vec_d):
            t = bcast.tile([P, H], bf16, tag="bcast")
            nc.gpsimd.dma_start(t[:], vec_d[:][None, :].to_broadcast((P, H)))
            return t

        def layer_norm_tile(x_f32_ap, gB, bB, out_bf):
            """x [128, H] fp32 -> out_bf [128, H] bf16 (LN with scale/shift)."""
            stats = lnp.tile([P, 2, nc.vector.BN_STATS_DIM], f32, tag="stats")
            xr = x_f32_ap.rearrange("p (n f) -> p n f", f=512)
            for i in range(2):
                nc.vector.bn_stats(out=stats[:, i, :], in_=xr[:, i, :])
            mv = lnp.tile([P, nc.vector.BN_AGGR_DIM], f32, tag="mv")
            nc.vector.bn_aggr(out=mv[:], in_=stats[:])
            rstd = lnp.tile([P, 1], f32, tag="rstd")
            nc.scalar.activation(out=rstd[:], in_=mv[:, 1:2], func=Act.Sqrt,
                                 bias=eps_t[:], scale=1.0)
            nc.vector.reciprocal(out=rstd[:], in_=rstd[:])
            t1 = lnp.tile([P, H], f32, tag="ln_tmp")
            nc.vector.tensor_scalar(out=t1[:], in0=x_f32_ap,
                                    scalar1=mv[:, 0:1], scalar2=rstd[:],
                                    op0=Alu.subtract, op1=Alu.mult)
            nc.vector.tensor_tensor(out=t1[:], in0=t1[:], in1=gB[:], op=Alu.mult)
            nc.vector.tensor_tensor(out=out_bf, in0=t1[:], in1=bB[:], op=Alu.add)

        def transpose_128(src_bf_ap, dst_bf_ap):
            pt = psA.tile([P, P], bf16, tag="psA")
            nc.tensor.transpose(pt[:], src_bf_ap, ident[:])
            nc.vector.tensor_copy(dst_bf_ap, pt[:])

        def load_w_tiled(dst, src_d):
            # per-kk-tile DMAs so accumulation chains can start on partial data
            for kk in range(KK):
                nc.sync.dma_start(
                    dst[:, kk, :],
                    src_d[kk * P:(kk + 1) * P, :])

        with tc.tile_pool(name="xsbp", bufs=4) as xsbp, \
             tc.tile_pool(name="qstr", bufs=2) as qstr:
            with tc.tile_pool(name="attops", bufs=1) as attops, \
                 tc.tile_pool(name="ctxp", bufs=4) as ctxp:
                qT = [attops.tile([P, T], bf16, tag=f"qT{j}", name=f"qT{j}")
                      for j in range(KK)]
                v_sb = [attops.tile([P, NH * (D + 1)], bf16, tag=f"v{s}",
                                    name=f"v{s}") for s in range(NST)]
                ctx = [ctxp.tile([P, H], bf16, tag="ctx", name=f"ctx{qt}")
                       for qt in range(NQT)]

                with tc.tile_pool(name="wp", bufs=2) as wp, \
                     tc.tile_pool(name="kvTp", bufs=8) as kvTp, \
                     tc.tile_pool(name="kTp", bufs=3) as kTp, \
                     tc.tile_pool(name="attnTp", bufs=10) as attnTp:
                    # query first (LN1 is the first consumer)
                    qsb = [qstr.tile([P, H], f32, tag="q_in", name=f"qin{qt}")
                           for qt in range(NQT)]
                    for qt in range(NQT):
                        nc.sync.dma_start(qsb[qt][:],
                                          q_d[qt * P:(qt + 1) * P, :])
                    nc.sync.dma_start(bq_t[:],
                                      bq_d[:].rearrange("(o p) -> p o", p=P))
                    nc.sync.dma_start(bk_t[:],
                                      bk_d[:].rearrange("(o p) -> p o", p=P))
                    g1B = bcast_tile(g1_d)
                    b1B = bcast_tile(b1_d)

                    wq_sb = wp.tile([P, KK, H], bf16, tag="w", name="wq_sb")
                    load_w_tiled(wq_sb, wq_d)
                    kvT_sb = [kvTp.tile([P, NKV], bf16, tag="kvT",
                                        name=f"kvT{kk}") for kk in range(KK)]
                    for kk in range(KK):
                        nc.sync.dma_start(kvT_sb[kk][:],
                                          kvT_d[kk * P:(kk + 1) * P, :])
                    wk_sb = wp.tile([P, KK, H], bf16, tag="w", name="wk_sb")
                    load_w_tiled(wk_sb, wk_d)
                    wv_sb = wp.tile([P, KK, H], bf16, tag="w", name="wv_sb")
                    load_w_tiled(wv_sb, wv_d)
                    bvB = bcast_tile(bv_d)

                    # ---- LN1(query) -> xn -> xnT ----
                    xnT = [xnTp.tile([P, T], bf16, tag="xnT", name=f"xnT{j}")
                           for j in range(KK)]
                    for qt in range(NQT):
                        xn = xnp.tile([P, H], bf16, tag="xn", name=f"xn{qt}")
                        layer_norm_tile(qsb[qt][:], g1B, b1B, xn[:])
                        for j in range(KK):
                            transpose_128(xn[:, j * P:(j + 1) * P],
                                          xnT[j][:, qt * P:(qt + 1) * P])

                    # ---- qT = Wq^T @ xnT + bq ----
                    for j in range(KK):
                        pq = psB.tile([P, T], f32, tag="psB", name=f"pq{j}")
                        for kk in range(KK):
                            nc.tensor.matmul(
                                pq[:], wq_sb[:, kk, j * P:(j + 1) * P],
                                xnT[kk][:], start=(kk == 0), stop=(kk == KK - 1))
                        nc.vector.tensor_scalar_add(out=qT[j][:], in0=pq[:],
                                                    scalar1=bq_t[:, j:j + 1])

                    def kproj(j, kt):
                        for c in range(4):
                            pk = psB.tile([P, T], f32, tag="psB",
                                          name=f"pk{j}_{c}")
                            for kk in range(KK):
                                nc.tensor.matmul(
                                    pk[:],
                                    wk_sb[:, kk, j * P:(j + 1) * P],
                                    kvT_sb[kk][:, c * 512:(c + 1) * 512],
                                    start=(kk == 0), stop=(kk == KK - 1))
                            nc.vector.tensor_scalar_add(
                                out=kt[:, c * 512:(c + 1) * 512], in0=pk[:],
                                scalar1=bk_t[:, j:j + 1])

                    def scores_half(jt, kt, h, half):
                        """exp(q_h . k / 8) for one head, kv tiles 8*half..+8."""
                        off = D * (h % 2)
                        hats = []
                        for gg in range(4):
                            g = 4 * half + gg
                            ps = psA.tile([P, 2, T], f32, tag="psA",
                                          name=f"ps{h}_{g}")
                            for s2 in range(2):
                                st = 2 * g + s2
                                nc.tensor.matmul(
                                    ps[:, s2, :],
                                    kt[off:off + D, st * P:(st + 1) * P],
                                    qT[jt][off:off + D, :],
                                    start=True, stop=True)
                            at = attnTp.tile([P, 2 * T], bf16, tag="attnT",
                                             name=f"at{h}_{g}")
                            hats.append(at)
                            nc.scalar.activation(
                                out=at[:],
                                in_=ps[:].rearrange("p a b -> p (a b)"),
                                func=Act.Exp, scale=0.125)
                        return hats

                    def ctx_full(h, hats8):
                        """v1-style: per qt, one 16-step chain + recip/scale."""
                        for qt in range(NQT):
                            pc = psB.tile([P, D + 1], f32, tag="psC",
                                          name=f"pc{h}_{qt}")
                            for st in range(NST):
                                vv = v_sb[st][:].rearrange(
                                    "p (hh x) -> p hh x", x=D + 1)
                                nc.tensor.matmul(
                                    pc[:],
                                    hats8[st // 2][:, (st % 2) * T + qt * P:
                                                   (st % 2) * T + (qt + 1) * P],
                                    vv[:, h, :],
                                    start=(st == 0), stop=(st == NST - 1))
                            rec = lnp.tile([P, 1], f32, tag="rec",
                                           name=f"rec{h}_{qt}")
                            nc.vector.reciprocal(out=rec[:], in_=pc[:, D:D + 1])
                            nc.vector.tensor_scalar_mul(
                                out=ctx[qt][:, h * D:(h + 1) * D],
                                in0=pc[:, :D], scalar1=rec[:])

                    def head_attention(jt, kt, h):
                        hats = scores_half(jt, kt, h, 0)
                        hats += scores_half(jt, kt, h, 1)
                        ctx_full(h, hats)

                    def vproj_half(half):
                        for st in range(8 * half, 8 * half + 8):
                            vt = v_sb[st][:].rearrange("p (h x) -> p h x",
                                                       x=D + 1)
                            nc.vector.memset(vt[:, :, D], 1.0)
                            for c in range(2):
                                pv = psB.tile([P, T], f32, tag="psB",
                                              name=f"pv{st}_{c}")
                                for kk in range(KK):
                                    nc.tensor.matmul(
                                        pv[:],
                                        kvT_sb[kk][:, st * P:(st + 1) * P],
                                        wv_sb[:, kk, c * 512:(c + 1) * 512],
                                        start=(kk == 0), stop=(kk == KK - 1))
                                nc.vector.tensor_tensor(
                                    out=vt[:, c * 8:(c + 1) * 8, :D],
                                    in0=pv[:].rearrange("p (a b) -> p a b", b=D),
                                    in1=bvB[:, c * 512:(c + 1) * 512].rearrange(
                                        "p (a b) -> p a b", b=D),
                                    op=Alu.add)

                    # ---- interleaved: first head's scores come before v-proj
                    #      so ACT exp work starts early ----
                    kt = kTp.tile([P, NKV], bf16, tag="kT", name="kT0")
                    kproj(0, kt)
                    hats00 = scores_half(0, kt, 0, 0)
                    vproj_half(0)
                    hats01 = scores_half(0, kt, 0, 1)
                    vproj_half(1)
                    ctx_full(0, hats00 + hats01)
                    head_attention(0, kt, 1)
                    for jt in range(1, KK):
                        kt = kTp.tile([P, NKV], bf16, tag="kT", name=f"kT{jt}")
                        kproj(jt, kt)
                        head_attention(jt, kt, 2 * jt)
                        head_attention(jt, kt, 2 * jt + 1)

                # ---- ctx transpose (reuses xnT slots) ----
                ctxT = [xnTp.tile([P, T], bf16, tag="xnT", name=f"ctxT{j}")
                        for j in range(KK)]
                for qt in range(NQT):
                    for j in range(KK):
                        transpose_128(ctx[qt][:, j * P:(j + 1) * P],
                                      ctxT[j][:, qt * P:(qt + 1) * P])

            # ---- o-proj + residual, LN2, MoE (with weight prefetch) ----
            with tc.tile_pool(name="wop", bufs=1) as wop, \
                 tc.tile_pool(name="gupp", bufs=2) as gupp, \
                 tc.tile_pool(name="dwnp", bufs=2) as dwnp, \
                 tc.tile_pool(name="outp", bufs=1) as outp, \
                 tc.tile_pool(name="moeact", bufs=2) as moeact, \
                 tc.tile_pool(name="interTp", bufs=16) as interTp:
                wo_sb = wop.tile([P, KK, H], bf16, tag="wo", name="wo_sb")
                load_w_tiled(wo_sb, wo_d)
                boB = bcast_tile(bo_d)
                g2B = bcast_tile(g2_d)
                b2B = bcast_tile(b2_d)
                qs2 = [qstr.tile([P, H], f32, tag="q_in", name=f"qin2_{qt}")
                       for qt in range(NQT)]
                for qt in range(NQT):
                    nc.sync.dma_start(qs2[qt][:], q_d[qt * P:(qt + 1) * P, :])
                # prefetch experts 0/1 weights on the SWDGE queue so the
                # latency-critical wo/q loads above are not blocked behind them
                gup_sbs = [gupp.tile([P, KK, 2 * I], bf16, tag="gup",
                                     name=f"gup{e}") for e in range(2)]
                dwn_sbs = [dwnp.tile([P, KK, H], bf16, tag="dwn",
                                     name=f"dwn{e}") for e in range(2)]
                for e in range(2):
                    for kk in range(KK):
                        nc.gpsimd.dma_start(gup_sbs[e][:, kk, :],
                                            gup_d[e, kk * P:(kk + 1) * P, :])
                        nc.gpsimd.dma_start(dwn_sbs[e][:, kk, :],
                                            dwn_d[e, kk * P:(kk + 1) * P, :])

                x_sb = [xsbp.tile([P, H], f32, tag="x", name=f"x{qt}")
                        for qt in range(NQT)]
                for qt in range(NQT):
                    for c in range(2):
                        po = psB.tile([P, T], f32, tag="psB",
                                      name=f"po{qt}_{c}")
                        for kk in range(KK):
                            nc.tensor.matmul(
                                po[:], ctxT[kk][:, qt * P:(qt + 1) * P],
                                wo_sb[:, kk, c * 512:(c + 1) * 512],
                                start=(kk == 0), stop=(kk == KK - 1))
                        sl = slice(c * 512, (c + 1) * 512)
                        nc.vector.tensor_tensor(out=x_sb[qt][:, sl], in0=po[:],
                                                in1=qs2[qt][:, sl], op=Alu.add)
                        nc.vector.tensor_tensor(out=x_sb[qt][:, sl],
                                                in0=x_sb[qt][:, sl],
                                                in1=boB[:, sl], op=Alu.add)

                # ---- LN2 -> xn2T (token tile == expert; reuses xnT slots) ----
                xn2T = [xnTp.tile([P, T], bf16, tag="xnT", name=f"xn2T{j}")
                        for j in range(KK)]
                for qt in range(NQT):
                    xn2 = xnp.tile([P, H], bf16, tag="xn", name=f"xn2_{qt}")
                    layer_norm_tile(x_sb[qt][:], g2B, b2B, xn2[:])
                    for j in range(KK):
                        transpose_128(xn2[:, j * P:(j + 1) * P],
                                      xn2T[j][:, qt * P:(qt + 1) * P])

                # ---- MoE (expert e <-> token tile e) ----
                out_sb = outp.tile([P, NQT, H], f32, tag="out")
                for e in range(E):
                    if e < 2:
                        gup_sb, dwn_sb = gup_sbs[e], dwn_sbs[e]
                    else:
                        gup_sb = gupp.tile([P, KK, 2 * I], bf16, tag="gup",
                                           name=f"gup{e}")
                        dwn_sb = dwnp.tile([P, KK, H], bf16, tag="dwn",
                                           name=f"dwn{e}")
                        for kk in range(KK):
                            nc.gpsimd.dma_start(gup_sb[:, kk, :],
                                                gup_d[e, kk * P:(kk + 1) * P, :])
                            nc.gpsimd.dma_start(dwn_sb[:, kk, :],
                                                dwn_d[e, kk * P:(kk + 1) * P, :])

                    sg = moeact.tile([P, I], bf16, tag="sg", name=f"sg{e}")
                    inter = moeact.tile([P, I], bf16, tag="inter",
                                        name=f"inter{e}")
                    for c in range(4):  # 512-wide chunks of 2I
                        pg = psB.tile([P, T], f32, tag="psB", name=f"pg{e}_{c}")
                        for kk in range(KK):
                            nc.tensor.matmul(
                                pg[:], xn2T[kk][:, e * P:(e + 1) * P],
                                gup_sb[:, kk, c * 512:(c + 1) * 512],
                                start=(kk == 0), stop=(kk == KK - 1))
                        if c < 2:  # gate chunk -> silu
                            nc.scalar.activation(
                                out=sg[:, c * 512:(c + 1) * 512], in_=pg[:],
                                func=Act.Silu)
                        else:  # up chunk -> inter = silu(gate) * up
                            sl = slice((c - 2) * 512, (c - 1) * 512)
                            nc.vector.tensor_tensor(out=inter[:, sl], in0=pg[:],
                                                    in1=sg[:, sl], op=Alu.mult)
                    interT = [interTp.tile([P, P], bf16, tag="interT",
                                           name=f"iT{e}_{ii}")
                              for ii in range(KK)]
                    for ii in range(KK):
                        transpose_128(inter[:, ii * P:(ii + 1) * P],
                                      interT[ii][:])
                    for c in range(2):
                        pd = psB.tile([P, T], f32, tag="psB", name=f"pd{e}_{c}")
                        for ii in range(KK):
                            nc.tensor.matmul(
                                pd[:], interT[ii][:],
                                dwn_sb[:, ii, c * 512:(c + 1) * 512],
                                start=(ii == 0), stop=(ii == KK - 1))
                        sl = slice(c * 512, (c + 1) * 512)
                        nc.vector.tensor_tensor(out=out_sb[:, e, sl],
                                                in0=pd[:],
                                                in1=x_sb[e][:, sl], op=Alu.add)
                    nc.sync.dma_start(out_d[e * P:(e + 1) * P, :],
                                      out_sb[:, e, :])

    nc.compile()
    return nc


def _get_program():
    if "nc" not in _BUILT:
        _BUILT["nc"] = _build_program()
    return _BUILT["nc"]


_PERM = np.array([l for r in range(E) for l in range(r, T, E)], dtype=np.int64)


def _make_in_maps(inputs):
    bf = ml_dtypes.bfloat16
    f = {k: np.ascontiguousarray(np.asarray(v, dtype=np.float32))
         for k, v in inputs.items()}
    shared = {
        "wq": f["Wq"].astype(bf), "wk": f["Wk"].astype(bf),
        "wv": f["Wv"].astype(bf), "wo": f["Wo"].astype(bf),
        "bq": f["bq"], "bk": f["bk"], "bv": f["bv"], "bo": f["bo"],
        "g1": f["g1"], "b1": f["b1"], "g2": f["g2"], "b2": f["b2"],
        "gup": f["gate_up"].astype(bf),
        "dwn": f["down"].astype(bf),
    }
    kvTs = [np.ascontiguousarray(f["key_value"][b].T).astype(bf)
            for b in range(B)]
    in_maps = []
    for c in range(8):
        b, hf = c // 2, c % 2
        qs = np.ascontiguousarray(f["query"][b, hf * T:(hf + 1) * T][_PERM])
        in_maps.append({"q": qs, "kvT": kvTs[b], **shared})
    return in_maps


def kernel(**inputs):
    from concourse.bass_utils import run_bass_kernel_spmd

    nc = _get_program()
    in_maps = _make_in_maps(inputs)
    res = run_bass_kernel_spmd(nc, in_maps, list(range(8)))

    out = np.empty((B, NQ, H), dtype=np.float32)
    for c in range(8):
        b, hf = c // 2, c % 2
        out[b, hf * T + _PERM] = res.results[c]["out"]
    return out


# revision 18
# speedup vs baseline: 175.1219x; 175.1219x over previous
"""Trainium2 Bass kernel for a cross-attention block with position-routed MoE.

Contract: kernel(**inputs) takes the FULL fp32 inputs (as produced by
setup_inputs) and returns the FULL [4, 1024, 1024] fp32 output.

Sharding (8 cores): core c handles batch b = c//2 and query-token half
h = c%2 (512 tokens). Tokens are permuted on the host so they are grouped
by position%4; MoE expert e then corresponds to contiguous token tile e.
KV projections are recomputed per half (duplicated across the 2 cores of a
batch) to avoid any cross-core communication.

Schedule: attention head-pairs are interleaved with the k-projection so the
ScalarE exp work overlaps the PE-dense projection stream; MoE weights are
prefetched at the attention/MoE boundary.
"""

import sys

if "/opt/trn_rl_repo" not in sys.path:
    sys.path.insert(0, "/opt/trn_rl_repo")

import numpy as np
import ml_dtypes

B = 4
NQ = 1024
NKV = 2048
H = 1024
NH = 16
D = 64  # head dim
E = 4
I = 1024  # expert intermediate
T = 512  # q tokens per core
P = 128
EPS = 1e-6
KK = H // P  # 8 contraction tiles
NST = NKV // P  # 16 kv-token tiles
NQT = T // P  # 4 q-token tiles

_BUILT = {}


def _build_program():
    from contextlib import ExitStack

    from concourse import bacc
    import concourse.mybir as mybir
    import concourse.tile as tile
    from concourse.masks import make_identity

    bf16 = mybir.dt.bfloat16
    f32 = mybir.dt.float32
    Alu = mybir.AluOpType
    Act = mybir.ActivationFunctionType

    nc = bacc.Bacc("TRN2", target_bir_lowering=False, debug=False, num_devices=8)

    # ---- DRAM I/O ----
    q_d = nc.dram_tensor("q", [T, H], f32, kind="ExternalInput")
    kvT_d = nc.dram_tensor("kvT", [H, NKV], bf16, kind="ExternalInput")
    wq_d = nc.dram_tensor("wq", [H, H], bf16, kind="ExternalInput")
    wk_d = nc.dram_tensor("wk", [H, H], bf16, kind="ExternalInput")
    wv_d = nc.dram_tensor("wv", [H, H], bf16, kind="ExternalInput")
    wo_d = nc.dram_tensor("wo", [H, H], bf16, kind="ExternalInput")
    bq_d = nc.dram_tensor("bq", [H], f32, kind="ExternalInput")
    bk_d = nc.dram_tensor("bk", [H], f32, kind="ExternalInput")
    bv_d = nc.dram_tensor("bv", [H], f32, kind="ExternalInput")
    bo_d = nc.dram_tensor("bo", [H], f32, kind="ExternalInput")
    g1_d = nc.dram_tensor("g1", [H], f32, kind="ExternalInput")
    b1_d = nc.dram_tensor("b1", [H], f32, kind="ExternalInput")
    g2_d = nc.dram_tensor("g2", [H], f32, kind="ExternalInput")
    b2_d = nc.dram_tensor("b2", [H], f32, kind="ExternalInput")
    gup_d = nc.dram_tensor("gup", [E, H, 2 * I], bf16, kind="ExternalInput")
    dwn_d = nc.dram_tensor("dwn", [E, I, H], bf16, kind="ExternalInput")
    out_d = nc.dram_tensor("out", [T, H], f32, kind="ExternalOutput")

    with tile.TileContext(nc) as tc, ExitStack() as stk:
        # ---- persistent pools (~22 KB/partition) ----
        consts = stk.enter_context(tc.tile_pool(name="consts", bufs=1))
        bcast = stk.enter_context(tc.tile_pool(name="bcast", bufs=3))
        lnp = stk.enter_context(tc.tile_pool(name="lnp", bufs=2))
        xnp = stk.enter_context(tc.tile_pool(name="xnp", bufs=2))
        xnTp = stk.enter_context(tc.tile_pool(name="xnT", bufs=8))
        psA = stk.enter_context(tc.tile_pool(name="psA", bufs=2, space="PSUM"))
        psB = stk.enter_context(tc.tile_pool(name="psB", bufs=2, space="PSUM"))

        ident = consts.tile([P, P], bf16, tag="ident")
        make_identity(nc, ident)
        eps_t = consts.tile([P, 1], f32, tag="eps")
        nc.vector.memset(eps_t, EPS)
        bq_t = consts.tile([P, KK], f32, tag="bq")
        bk_t = consts.tile([P, KK], f32, tag="bk")

        def bcast_tile(vec_d):
            t = bcast.tile([P, H], bf16, tag="bcast")
            nc.gpsimd.dma_start(t[:], vec_d[:][None, :].to_broadcast((P, H)))
            return t

        def layer_norm_tile(x_f32_ap, gB, bB, out_bf):
            """x [128, H] fp32 -> out_bf [128, H] bf16 (LN with scale/shift)."""
            stats = lnp.tile([P, 2, nc.vector.BN_STATS_DIM], f32, tag="stats")
            xr = x_f32_ap.rearrange("p (n f) -> p n f", f=512)
            for i in range(2):
                nc.vector.bn_stats(out=stats[:, i, :], in_=xr[:, i, :])
            mv = lnp.tile([P, nc.vector.BN_AGGR_DIM], f32, tag="mv")
            nc.vector.bn_aggr(out=mv[:], in_=stats[:])
            rstd = lnp.tile([P, 1], f32, tag="rstd")
            nc.scalar.activation(out=rstd[:], in_=mv[:, 1:2], func=Act.Sqrt,
                                 bias=eps_t[:], scale=1.0)
            nc.vector.reciprocal(out=rstd[:], in_=rstd[:])
            t1 = lnp.tile([P, H], f32, tag="ln_tmp")
            nc.vector.tensor_scalar(out=t1[:], in0=x_f32_ap,
                                    scalar1=mv[:, 0:1], scalar2=rstd[:],
                                    op0=Alu.subtract, op1=Alu.mult)
            nc.vector.tensor_tensor(out=t1[:], in0=t1[:], in1=gB[:], op=Alu.mult)
            nc.vector.tensor_tensor(out=out_bf, in0=t1[:], in1=bB[:], op=Alu.add)

        def transpose_128(src_bf_ap, dst_bf_ap):
            pt = psA.tile([P, P], bf16, tag="psA")
            nc.tensor.transpose(pt[:], src_bf_ap, ident[:])
            nc.vector.tensor_copy(dst_bf_ap, pt[:])

        def load_w_tiled(dst, src_d):
            # per-kk-tile DMAs so accumulation chains can start on partial data
            for kk in range(KK):
                nc.sync.dma_start(
                    dst[:, kk, :],
                    src_d[kk * P:(kk + 1) * P, :])

        with tc.tile_pool(name="xsbp", bufs=4) as xsbp, \
             tc.tile_pool(name="qstr", bufs=2) as qstr:
            with tc.tile_pool(name="attops", bufs=1) as attops, \
                 tc.tile_pool(name="ctxp", bufs=4) as ctxp:
                qT = [attops.tile([P, T], bf16, tag=f"qT{j}", name=f"qT{j}")
                      for j in range(KK)]
                v_sb = [attops.tile([P, NH * (D + 1)], bf16, tag=f"v{s}",
                                    name=f"v{s}") for s in range(NST)]
                ctx = [ctxp.tile([P, H], bf16, tag="ctx", name=f"ctx{qt}")
                       for qt in range(NQT)]

                with tc.tile_pool(name="wp", bufs=2) as wp, \
                     tc.tile_pool(name="kvTp", bufs=8) as kvTp, \
                     tc.tile_pool(name="kTp", bufs=3) as kTp, \
                     tc.tile_pool(name="attnTp", bufs=10) as attnTp:
                    # query first (LN1 is the first consumer)
                    qsb = [qstr.tile([P, H], f32, tag="q_in", name=f"qin{qt}")
                           for qt in range(NQT)]
                    for qt in range(NQT):
                        nc.sync.dma_start(qsb[qt][:],
                                          q_d[qt * P:(qt + 1) * P, :])
                    nc.sync.dma_start(bq_t[:],
                                      bq_d[:].rearrange("(o p) -> p o", p=P))
                    nc.sync.dma_start(bk_t[:],
                                      bk_d[:].rearrange("(o p) -> p o", p=P))
                    g1B = bcast_tile(g1_d)
                    b1B = bcast_tile(b1_d)

                    wq_sb = wp.tile([P, KK, H], bf16, tag="w", name="wq_sb")
                    load_w_tiled(wq_sb, wq_d)
                    kvT_sb = [kvTp.tile([P, NKV], bf16, tag="kvT",
                                        name=f"kvT{kk}") for kk in range(KK)]
                    for kk in range(KK):
                        nc.sync.dma_start(kvT_sb[kk][:],
                                          kvT_d[kk * P:(kk + 1) * P, :])
                    wk_sb = wp.tile([P, KK, H], bf16, tag="w", name="wk_sb")
                    load_w_tiled(wk_sb, wk_d)
                    wv_sb = wp.tile([P, KK, H], bf16, tag="w", name="wv_sb")
                    load_w_tiled(wv_sb, wv_d)
                    bvB = bcast_tile(bv_d)

                    # ---- LN1(query) -> xn -> xnT ----
                    xnT = [xnTp.tile([P, T], bf16, tag="xnT", name=f"xnT{j}")
                           for j in range(KK)]
                    for qt in range(NQT):
                        xn = xnp.tile([P, H], bf16, tag="xn", name=f"xn{qt}")
                        layer_norm_tile(qsb[qt][:], g1B, b1B, xn[:])
                        for j in range(KK):
                            transpose_128(xn[:, j * P:(j + 1) * P],
                                          xnT[j][:, qt * P:(qt + 1) * P])

                    # ---- qT = Wq^T @ xnT + bq ----
                    for j in range(KK):
                        pq = psB.tile([P, T], f32, tag="psB", name=f"pq{j}")
                        for kk in range(KK):
                            nc.tensor.matmul(
                                pq[:], wq_sb[:, kk, j * P:(j + 1) * P],
                                xnT[kk][:], start=(kk == 0), stop=(kk == KK - 1))
                        nc.vector.tensor_scalar_add(out=qT[j][:], in0=pq[:],
                                                    scalar1=bq_t[:, j:j + 1])

                    def kproj(j, kt):
                        for c in range(4):
                            pk = psB.tile([P, T], f32, tag="psB",
                                          name=f"pk{j}_{c}")
                            for kk in range(KK):
                                nc.tensor.matmul(
                                    pk[:],
                                    wk_sb[:, kk, j * P:(j + 1) * P],
                                    kvT_sb[kk][:, c * 512:(c + 1) * 512],
                                    start=(kk == 0), stop=(kk == KK - 1))
                            nc.vector.tensor_scalar_add(
                                out=kt[:, c * 512:(c + 1) * 512], in0=pk[:],
                                scalar1=bk_t[:, j:j + 1])

                    def scores_half(jt, kt, h, half):
                        """exp(q_h . k / 8) for one head, kv tiles 8*half..+8."""
                        off = D * (h % 2)
                        hats = []
                        for gg in range(4):
                            g = 4 * half + gg
                            ps = psA.tile([P, 2, T], f32, tag="psA",
                                          name=f"ps{h}_{g}")
                            for s2 in range(2):
                                st = 2 * g + s2
                                nc.tensor.matmul(
                                    ps[:, s2, :],
                                    kt[off:off + D, st * P:(st + 1) * P],
                                    qT[jt][off:off + D, :],
                                    start=True, stop=True)
                            at = attnTp.tile([P, 2 * T], bf16, tag="attnT",
                                             name=f"at{h}_{g}")
                            hats.append(at)
                            nc.scalar.activation(
                                out=at[:],
                                in_=ps[:].rearrange("p a b -> p (a b)"),
                                func=Act.Exp, scale=0.125)
                        return hats

                    def ctx_full(h, hats8):
                        """v1-style: per qt, one 16-step chain + recip/scale."""
                        for qt in range(NQT):
                            pc = psB.tile([P, D + 1], f32, tag="psC",
                                          name=f"pc{h}_{qt}")
                            for st in range(NST):
                                vv = v_sb[st][:].rearrange(
                                    "p (hh x) -> p hh x", x=D + 1)
                                nc.tensor.matmul(
                                    pc[:],
                                    hats8[st // 2][:, (st % 2) * T + qt * P:
                                                   (st % 2) * T + (qt + 1) * P],
                                    vv[:, h, :],
                                    start=(st == 0), stop=(st == NST - 1))
                            rec = lnp.tile([P, 1], f32, tag="rec",
                                           name=f"rec{h}_{qt}")
                            nc.vector.reciprocal(out=rec[:], in_=pc[:, D:D + 1])
                            nc.vector.tensor_scalar_mul(
                                out=ctx[qt][:, h * D:(h + 1) * D],
                                in0=pc[:, :D], scalar1=rec[:])

                    def head_attention(jt, kt, h):
                        hats = scores_half(jt, kt, h, 0)
                        hats += scores_half(jt, kt, h, 1)
                        ctx_full(h, hats)

                    def vproj_half(half):
                        for st in range(8 * half, 8 * half + 8):
                            vt = v_sb[st][:].rearrange("p (h x) -> p h x",
                                                       x=D + 1)
                            nc.vector.memset(vt[:, :, D], 1.0)
                            for c in range(2):
                                pv = psB.tile([P, T], f32, tag="psB",
                                              name=f"pv{st}_{c}")
                                for kk in range(KK):
                                    nc.tensor.matmul(
                                        pv[:],
                                        kvT_sb[kk][:, st * P:(st + 1) * P],
                                        wv_sb[:, kk, c * 512:(c + 1) * 512],
                                        start=(kk == 0), stop=(kk == KK - 1))
                                nc.vector.tensor_tensor(
                                    out=vt[:, c * 8:(c + 1) * 8, :D],
                                    in0=pv[:].rearrange("p (a b) -> p a b", b=D),
                                    in1=bvB[:, c * 512:(c + 1) * 512].rearrange(
                                        "p (a b) -> p a b", b=D),
                                    op=Alu.add)

                    # ---- interleaved: first head's scores come before v-proj
                    #      so ACT exp work starts early ----
                    kt = kTp.tile([P, NKV], bf16, tag="kT", name="kT0")
                    kproj(0, kt)
                    hats00 = scores_half(0, kt, 0, 0)
                    vproj_half(0)
                    hats01 = scores_half(0, kt, 0, 1)
                    vproj_half(1)
                    ctx_full(0, hats00 + hats01)
                    head_attention(0, kt, 1)
                    for jt in range(1, KK):
                        kt = kTp.tile([P, NKV], bf16, tag="kT", name=f"kT{jt}")
                        kproj(jt, kt)
                        head_attention(jt, kt, 2 * jt)
                        head_attention(jt, kt, 2 * jt + 1)

                # ---- ctx transpose (reuses xnT slots) ----
                ctxT = [xnTp.tile([P, T], bf16, tag="xnT", name=f"ctxT{j}")
                        for j in range(KK)]
                for qt in range(NQT):
                    for j in range(KK):
                        transpose_128(ctx[qt][:, j * P:(j + 1) * P],
                                      ctxT[j][:, qt * P:(qt + 1) * P])

            # ---- o-proj + residual, LN2, MoE (with weight prefetch) ----
            with tc.tile_pool(name="wop", bufs=1) as wop, \
                 tc.tile_pool(name="gupp", bufs=2) as gupp, \
                 tc.tile_pool(name="dwnp", bufs=2) as dwnp, \
                 tc.tile_pool(name="outp", bufs=1) as outp, \
                 tc.tile_pool(name="moeact", bufs=2) as moeact, \
                 tc.tile_pool(name="interTp", bufs=16) as interTp:
                wo_sb = wop.tile([P, KK, H], bf16, tag="wo", name="wo_sb")
                load_w_tiled(wo_sb, wo_d)
                boB = bcast_tile(bo_d)
                g2B = bcast_tile(g2_d)
                b2B = bcast_tile(b2_d)
                qs2 = [qstr.tile([P, H], f32, tag="q_in", name=f"qin2_{qt}")
                       for qt in range(NQT)]
                for qt in range(NQT):
                    nc.sync.dma_start(qs2[qt][:], q_d[qt * P:(qt + 1) * P, :])
                # prefetch experts 0/1 weights on the SWDGE queue so the
                # latency-critical wo/q loads above are not blocked behind them
                gup_sbs = [gupp.tile([P, KK, 2 * I], bf16, tag="gup",
                                     name=f"gup{e}") for e in range(2)]
                dwn_sbs = [dwnp.tile([P, KK, H], bf16, tag="dwn",
                                     name=f"dwn{e}") for e in range(2)]
                for e in range(2):
                    for kk in range(KK):
                        nc.gpsimd.dma_start(gup_sbs[e][:, kk, :],
                                            gup_d[e, kk * P:(kk + 1) * P, :])
                        nc.gpsimd.dma_start(dwn_sbs[e][:, kk, :],
                                            dwn_d[e, kk * P:(kk + 1) * P, :])

                x_sb = [xsbp.tile([P, H], f32, tag="x", name=f"x{qt}")
                        for qt in range(NQT)]
                for qt in range(NQT):
                    for c in range(2):
                        po = psB.tile([P, T], f32, tag="psB",
                                      name=f"po{qt}_{c}")
                        for kk in range(KK):
                            nc.tensor.matmul(
                                po[:], ctxT[kk][:, qt * P:(qt + 1) * P],
                                wo_sb[:, kk, c * 512:(c + 1) * 512],
                                start=(kk == 0), stop=(kk == KK - 1))
                        sl = slice(c * 512, (c + 1) * 512)
                        nc.vector.tensor_tensor(out=x_sb[qt][:, sl], in0=po[:],
                                                in1=qs2[qt][:, sl], op=Alu.add)
                        nc.vector.tensor_tensor(out=x_sb[qt][:, sl],
                                                in0=x_sb[qt][:, sl],
                                                in1=boB[:, sl], op=Alu.add)

                # ---- LN2 -> xn2T (token tile == expert; reuses xnT slots) ----
                xn2T = [xnTp.tile([P, T], bf16, tag="xnT", name=f"xn2T{j}")
                        for j in range(KK)]
                for qt in range(NQT):
                    xn2 = xnp.tile([P, H], bf16, tag="xn", name=f"xn2_{qt}")
                    layer_norm_tile(x_sb[qt][:], g2B, b2B, xn2[:])
                    for j in range(KK):
                        transpose_128(xn2[:, j * P:(j + 1) * P],
                                      xn2T[j][:, qt * P:(qt + 1) * P])

                # ---- MoE (expert e <-> token tile e) ----
                out_sb = outp.tile([P, NQT, H], f32, tag="out")
                for e in range(E):
                    if e < 2:
                        gup_sb, dwn_sb = gup_sbs[e], dwn_sbs[e]
                    else:
                        gup_sb = gupp.tile([P, KK, 2 * I], bf16, tag="gup",
                                           name=f"gup{e}")
                        dwn_sb = dwnp.tile([P, KK, H], bf16, tag="dwn",
                                           name=f"dwn{e}")
                        for kk in range(KK):
                            nc.gpsimd.dma_start(gup_sb[:, kk, :],
                                                gup_d[e, kk * P:(kk + 1) * P, :])
                            nc.gpsimd.dma_start(dwn_sb[:, kk, :],
                                                dwn_d[e, kk * P:(kk + 1) * P, :])

                    sg = moeact.tile([P, I], bf16, tag="sg", name=f"sg{e}")
                    inter = moeact.tile([P, I], bf16, tag="inter",
                                        name=f"inter{e}")
                    for c in range(4):  # 512-wide chunks of 2I
                        pg = psB.tile([P, T], f32, tag="psB", name=f"pg{e}_{c}")
                        for kk in range(KK):
                            nc.tensor.matmul(
                                pg[:], xn2T[kk][:, e * P:(e + 1) * P],
                                gup_sb[:, kk, c * 512:(c + 1) * 512],
                                start=(kk == 0), stop=(kk == KK - 1))
                        if c < 2:  # gate chunk -> silu
                            nc.scalar.activation(
                                out=sg[:, c * 512:(c + 1) * 512], in_=pg[:],
                                func=Act.Silu)
                        else:  # up chunk -> inter = silu(gate) * up
                            sl = slice((c - 2) * 512, (c - 1) * 512)
                            nc.vector.tensor_tensor(out=inter[:, sl], in0=pg[:],
                                                    in1=sg[:, sl], op=Alu.mult)
                    interT = [interTp.tile([P, P], bf16, tag="interT",
                                           name=f"iT{e}_{ii}")
                              for ii in range(KK)]
                    for ii in range(KK):
                        transpose_128(inter[:, ii * P:(ii + 1) * P],
                                      interT[ii][:])
                    for c in range(2):
                        pd = psB.tile([P, T], f32, tag="psB", name=f"pd{e}_{c}")
                        for ii in range(KK):
                            nc.tensor.matmul(
                                pd[:], interT[ii][:],
                                dwn_sb[:, ii, c * 512:(c + 1) * 512],
                                start=(ii == 0), stop=(ii == KK - 1))
                        sl = slice(c * 512, (c + 1) * 512)
                        nc.vector.tensor_tensor(out=out_sb[:, e, sl],
                                                in0=pd[:],
                                                in1=x_sb[e][:, sl], op=Alu.add)
                    nc.sync.dma_start(out_d[e * P:(e + 1) * P, :],
                                      out_sb[:, e, :])

    nc.compile()
    return nc


def _get_program():
    if "nc" not in _BUILT:
        _BUILT["nc"] = _build_program()
    return _BUILT["nc"]


_PERM = np.array([l for r in range(E) for l in range(r, T, E)], dtype=np.int64)


def _make_in_maps(inputs):
    bf = ml_dtypes.bfloat16
    f = {k: np.ascontiguousarray(np.asarray(v, dtype=np.float32))
         for k, v in inputs.items()}
    shared = {
        "wq": f["Wq"].astype(bf), "wk": f["Wk"].astype(bf),
        "wv": f["Wv"].astype(bf), "wo": f["Wo"].astype(bf),
        "bq": f["bq"], "bk": f["bk"], "bv": f["bv"], "bo": f["bo"],
        "g1": f["g1"], "b1": f["b1"], "g2": f["g2"], "b2": f["b2"],
        "gup": f["gate_up"].astype(bf),
        "dwn": f["down"].astype(bf),
    }
    kvTs = [np.ascontiguousarray(f["key_value"][b].T).astype(bf)
            for b in range(B)]
    in_maps = []
    for c in range(8):
        b, hf = c // 2, c % 2
        qs = np.ascontiguousarray(f["query"][b, hf * T:(hf + 1) * T][_PERM])
        in_maps.append({"q": qs, "kvT": kvTs[b], **shared})
    return in_maps


def kernel(**inputs):
    from concourse.bass_utils import run_bass_kernel_spmd

    nc = _get_program()
    in_maps = _make_in_maps(inputs)
    res = run_bass_kernel_spmd(nc, in_maps, list(range(8)))

    out = np.empty((B, NQ, H), dtype=np.float32)
    for c in range(8):
        b, hf = c // 2, c % 2
        out[b, hf * T + _PERM] = res.results[c]["out"]
    return out
